# revision 41
# baseline (speedup 1.0000x reference)
"""Trainium2 Bass kernel for nn_AttentionSiphon.

Reference computes: tokens = x @ W_map + b_map; concat [time, cluster, tokens];
LayerNorm; per-head q/k projections; softmax(q k^T / sqrt(dh)); mean over heads;
returns rows 0 and 1 of the [B,S,S] head-mean attention.

Only attention rows 0/1 are returned, and their queries come from the
(batch-independent) time/cluster tokens, so per-head attention collapses to

  score[j, c=2h+r] = LN(token_j) . (Wk[h] @ q_r[h])   (+ constants)

The 34 score/stat columns are LINEAR in x:  Y = Vaug^T (W^T x^T) = A^T x^T
with A = W @ Vaug [512, 34] precomputed on host.  Only the LayerNorm
sum-of-squares is quadratic:  SQ_j = ||W^T x_j||^2 = x_j^T (W W^T) x_j
= ||L^T x_j||^2 with L = cholesky(W W^T) [512, 512].  So the device work per
core (1024 token columns) is U = L^T x (512-contraction, half the FLOPs of the
naive 1024-wide token projection), squares+reduce for SQ, and the tiny A^T x.
L is lower-triangular, so of the 4x4 grid of [128,128] contraction blocks only
the kc >= dc ones are nonzero: 10 matmuls per 512-column tile instead of 16.

Device output per core: [34, 2, 1024] f32 — [:,0,:] = Y^T, [0,1,:] = SQ.
The tiny softmax epilogue runs on host (identical to the previous scheme).
"""

import os
import sys

sys.path.insert(0, "/opt/trn_rl_repo")

import numpy as np
import ml_dtypes

B, N, IN_D = 4, 2046, 512
D, H, DH = 1024, 16, 64
S = N + 2
EPS = 1e-5
NCORES = 8
JPC = 1024            # padded rows per core
JTOT = NCORES * JPC   # 8192 (8184 real rows + 8 pad)
NAUG = 34             # 32 score cols + colsum + b_map cross
NC_OUT = NAUG + 1     # + sumsq row

# Precision scheme: "bf16" (fastest, ~1.6e-3 rel err),
# "f32r" (fp32-storage reduced-precision matmuls at bf16 PE speed, ~2e-4)
PRECISION = os.environ.get("AS_PRECISION", "bf16")
WARMUP_MMS = int(os.environ.get("AS_WARM", "17"))

_PROG_CACHE = {}
LAST_RESULT = None  # BassKernelResults of the most recent run (for test harness)


def _bf16(a):
    return np.asarray(a, np.float32).astype(ml_dtypes.bfloat16)


def _build_program(precision, warmup=None):
    if warmup is None:
        warmup = WARMUP_MMS
    import concourse.bacc as bacc
    import concourse.mybir as mybir
    from concourse import tile
    from concourse.tile import ScopedClock

    class LeanTailTileContext(tile.TileContext):
        """Skip the exit-path double all-engine barrier + per-sem clears.

        The kernel preamble (Bass.__init__, target_bir_lowering) already
        dma_reset+sem_clears the kernel sem range at the start of every
        execution, and this program has a single TileContext, so nothing
        downstream consumes the freed sems. The final Sync drain still
        waits on every proc (incl. DMA lanes), so outputs are complete
        before the instruction streams end.
        """

        def _drain_and_barrier(self, tick_clock, wait_clock):
            drain_inst = self.nc.sync.drain()
            wait_clock.add_sem_waits(
                drain_inst.ins, ScopedClock({None: tick_clock.global_clock})
            )
            popped = self.nc._tile_sem_poison_stack.pop()
            assert popped is self._sem_poison

    f32 = mybir.dt.float32
    bf16 = mybir.dt.bfloat16
    AF = mybir.ActivationFunctionType

    nc = bacc.Bacc("TRN2")

    bf = mybir.dt.float32r if precision == "f32r" else bf16

    # L-blocks (kc>=dc, per dc in emission order dc=3,2,1,0) + A chunks,
    # all fused into one per-partition-contiguous tensor for a single
    # fat-packet DMA.  Column offsets precomputed here.
    DCS = [3, 2, 1, 0]
    lblk = {}
    col = 0
    for dc in DCS:
        for kc in range(dc, 4):
            lblk[(dc, kc)] = col
            col += 128
    acol = {}
    for kc in range(4):
        acol[kc] = col
        col += 32
    LWA_W = col  # 10*128 + 4*32 = 1408

    # Inputs fused into fat tensors (whole rows DMA'd at once) so each
    # partition row is one long contiguous DRAM region — short rows starve
    # the DMA engines on descriptor fetches (measured 58% vs 100% engine
    # busy).  Split by first-need: weights + x chunks kc3/kc2 unblock the
    # first U groups, kc1/kc0 the rest of jt0, then jt1.
    inA = nc.dram_tensor("inA", [128, LWA_W + 512], bf, kind="ExternalInput")
    inB = nc.dram_tensor("inB", [128, 1536], bf, kind="ExternalInput")
    inC = nc.dram_tensor("inC", [128, 2048], bf, kind="ExternalInput")
    # out[0:32, jt, :] = Y^T (32 scores); out[32, jt, :] = sumsq.  Y and SQ
    # share one [33, 512] PSUM tile per jt — the sumsq ones-matmuls target
    # partition 32 via tile_position=(0, 32) — so each jt needs a single
    # PSUM->SBUF copy, and one fat final DMA ships both jt halves.
    out_h = nc.dram_tensor("out", [33, 2, 512], f32, kind="ExternalOutput")

    ones_bf = nc.const_aps.tensor(1.0, [128, 1], bf16)

    with LeanTailTileContext(nc) as tc:
        with (
            tc.tile_pool(name="cst", bufs=1) as cst,
            tc.tile_pool(name="scr", bufs=2) as scr,
            tc.tile_pool(name="ps_u", bufs=5, space="PSUM") as ps_u,
            tc.tile_pool(name="ps_y", bufs=2, space="PSUM") as ps_y,
            tc.tile_pool(name="ps_w", bufs=1, space="PSUM") as ps_w,
        ):
            inA_sb = cst.tile([128, LWA_W + 512], bf, name="inA_sb",
                              tag="inA")
            inB_sb = cst.tile([128, 1536], bf, name="inB_sb", tag="inB")
            inC_sb = cst.tile([128, 2048], bf, name="inC_sb", tag="inC")
            out_sb = cst.tile([33, 2, 512], f32, name="out_sb")

            def lwa_sl(c, w):
                return inA_sb[:, c:c + w]

            def xt_sl(jt, kc):
                # jt0: kc3 rides with the weights in inA; kc2/kc1/kc0
                # follow in inB in first-need order; jt1 all in inC
                if jt == 1:
                    return inC_sb[:, kc * 512:(kc + 1) * 512]
                if kc == 3:
                    return inA_sb[:, LWA_W:LWA_W + 512]
                return inB_sb[:, (2 - kc) * 512:(3 - kc) * 512]

            # All input DMA on the Sync HWDGE ring: one ring at full rate
            # beats two shared ones, and the Scalar ring stalls ~1.5us
            # behind its activation-table load.
            nc.sync.dma_start(inA_sb[:], inA[:])
            nc.sync.dma_start(inB_sb[:], inB[:])
            nc.sync.dma_start(inC_sb[:], inC[:])

            # PE warm-up during the DMA fill: the HAM activity monitor only
            # un-throttles (1.2 -> 2.4 GHz) after ~3.4us of genuinely busy
            # PE; N=1 matmuls don't register, so stream N=256 ones off a
            # memset tile (baseline-style).
            if warmup:
                warm_sb = cst.tile([128, 256], bf16, name="warm_sb")
                nc.gpsimd.memset(warm_sb[:], 0.25)
                psw = ps_w.tile([128, 256], f32, name="psw", tag="psw")
                for _ in range(warmup):
                    nc.tensor.matmul(psw[:], warm_sb[:, 0:128], warm_sb[:],
                                     start=True, stop=True)

            for jt in range(2):
                # ---- U = L^T x (triangular: block dc needs kc>=dc) ----
                # dc=3 first (1 matmul) so its square lands early; the
                # sumsq ones-matmuls accumulate as squares become ready,
                # with Y before the last one so the PE never stalls.
                sq = {}
                for dc in DCS:
                    psu = ps_u.tile([128, 512], f32, name="psu", tag="psu")
                    kcs = list(range(dc, 4))
                    for ki, kc in enumerate(kcs):
                        nc.tensor.matmul(
                            psu[:],
                            lwa_sl(lblk[(dc, kc)], 128),
                            xt_sl(jt, kc),
                            start=(ki == 0),
                            stop=(ki == len(kcs) - 1),
                        )
                    # squared chunk (bf16; LN variance is error-tolerant)
                    sq_t = scr.tile([128, 512], bf16, name=f"sq{dc}",
                                    tag=f"sq{dc}")
                    nc.scalar.activation(sq_t[:], psu[:], AF.Square)
                    sq[dc] = sq_t

                py = ps_y.tile([33, 512], f32, name="py", tag="py")
                # sumsq partial sums into partition 32 as squares arrive
                for dc in [3, 2, 1]:
                    nc.tensor.matmul(py[32:33, :], ones_bf, sq[dc][:],
                                     start=(dc == 3), stop=False,
                                     tile_position=(0, 32))
                # ---- scores Y^T = A^T x into partitions 0..31 ----
                for kc in range(4):
                    nc.tensor.matmul(
                        py[0:32, :],
                        lwa_sl(acol[kc], 32),
                        xt_sl(jt, kc),
                        start=(kc == 0),
                        stop=(kc == 3),
                    )
                # last sumsq chunk lands while Y streams
                nc.tensor.matmul(py[32:33, :], ones_bf, sq[0][:],
                                 start=False, stop=True,
                                 tile_position=(0, 32))

                nc.vector.tensor_copy(out_sb[:, jt, :], py[:])

            # one fat-row DMA for both halves (thin per-jt slices starve
            # the DMA engines; the jt0 half just waits for jt1's copy)
            nc.sync.dma_start(out_h[:], out_sb[:])

    nc.compile()
    return nc


def _host_precompute(inputs):
    x = np.asarray(inputs["x"], np.float32)
    W = np.asarray(inputs["W_map"], np.float32)
    b_map = np.asarray(inputs["b_map"], np.float32)
    g = np.asarray(inputs["ln_g"], np.float32)
    lb = np.asarray(inputs["ln_b"], np.float32)
    Wq = np.asarray(inputs["Wq"], np.float32)
    bq = np.asarray(inputs["bq"], np.float32)
    Wk = np.asarray(inputs["Wk"], np.float32)
    bk = np.asarray(inputs["bk"], np.float32)
    tt = np.asarray(inputs["time_token"], np.float32)
    ct = np.asarray(inputs["cluster_token"], np.float32)

    spec = np.concatenate([tt, ct], 0)                      # [2, D]
    mu = spec.mean(-1, keepdims=True)
    var = ((spec - mu) ** 2).mean(-1, keepdims=True)
    hspec = ((spec - mu) / np.sqrt(var + EPS) * g + lb).reshape(2, H, DH)
    q = np.einsum("rhd,hde->rhe", hspec, Wq) + bq[None]
    qs = (q / np.sqrt(DH)).astype(np.float32)               # [2,H,DH]
    kspec = np.einsum("rhd,hde->rhe", hspec, Wk) + bk[None]
    s_spec = np.einsum("rhe,the->hrt", qs, kspec)           # [H,2,2]

    v = np.einsum("hde,rhe->hdr", Wk, qs)                   # [H,DH,2]
    V = np.zeros((D, 2 * H), np.float32)
    for h in range(H):
        V[64 * h:64 * h + 64, 2 * h] = v[h, :, 0]
        V[64 * h:64 * h + 64, 2 * h + 1] = v[h, :, 1]
    c0 = np.empty(2 * H, np.float32)
    for h in range(H):
        c0[2 * h] = qs[0, h] @ bk[h]
        c0[2 * h + 1] = qs[1, h] @ bk[h]

    Vg = g[:, None] * V
    consts = dict(
        pg=Vg.sum(0),
        qb=(lb[:, None] * V).sum(0),
        bVg=(b_map[:, None] * Vg).sum(0),
        bmean=b_map.mean(),
        bsq=(b_map ** 2).sum(),
        s_spec=s_spec,
        c0=c0,
        # colsum/bcross are linear in x with tiny [512] maps — cheaper and
        # more accurate on host than as extra device score columns
        wc=(W @ np.ones(D, np.float32)).astype(np.float32),
        bc=(W @ b_map).astype(np.float32),
    )

    # collapse the linear part through W; factor the quadratic part
    W64 = W.astype(np.float64)
    A = (W64 @ Vg.astype(np.float64)).astype(np.float32)    # [512, 32]
    L = np.linalg.cholesky(W64 @ W64.T).astype(np.float32)  # [512, 512]
    return x, A, L, consts


def kernel(**inputs):
    from concourse.bass_utils import run_bass_kernel_spmd

    x, A, L, consts = _host_precompute(inputs)

    key = (PRECISION, WARMUP_MMS)
    if key not in _PROG_CACHE:
        _PROG_CACHE[key] = _build_program(PRECISION, WARMUP_MMS)
    nc = _PROG_CACHE[key]

    cast = (lambda a: np.asarray(a, np.float32)) if PRECISION == "f32r" \
        else _bf16

    xf = x.reshape(B * N, IN_D)
    xpad = np.zeros((JTOT, IN_D), np.float32)
    xpad[:B * N] = xf

    # fused L-blocks (kc>=dc, dc order 3,2,1,0) + A chunks: [128, 1408];
    # x^T jt0's kc3 chunk rides along in inA (fat DMA rows)
    LWA_W = 10 * 128 + 4 * 32
    lwa_np = np.empty((128, LWA_W), np.float32)
    col = 0
    for dc in [3, 2, 1, 0]:
        for kc in range(dc, 4):
            lwa_np[:, col:col + 128] = \
                L[kc * 128:(kc + 1) * 128, dc * 128:(dc + 1) * 128]
            col += 128
    for kc in range(4):
        lwa_np[:, col:col + 32] = A[kc * 128:(kc + 1) * 128, :]
        col += 32
    lwa_c = cast(lwa_np)

    in_maps = []
    for c in range(NCORES):
        xT = np.ascontiguousarray(xpad[c * JPC:(c + 1) * JPC].T)  # [512,1024]
        # [512, 1024] -> [128p, 2jt, 4kc, 512]
        xp = cast(xT).reshape(4, 128, 2, 512).transpose(1, 2, 0, 3)
        ia = np.empty((128, LWA_W + 512), lwa_c.dtype)
        ia[:, :LWA_W] = lwa_c
        ia[:, LWA_W:] = xp[:, 0, 3]                       # jt0 kc3
        ib = np.empty((128, 1536), lwa_c.dtype)
        for kc in (2, 1, 0):                              # first-need order
            ib[:, (2 - kc) * 512:(3 - kc) * 512] = xp[:, 0, kc]
        m = {"inA": np.ascontiguousarray(ia),
             "inB": np.ascontiguousarray(ib),
             "inC": np.ascontiguousarray(xp[:, 1].reshape(128, 2048))}
        in_maps.append(m)

    trace = bool(int(os.environ.get("AS_TRACE", "0")))
    res = run_bass_kernel_spmd(nc, in_maps, list(range(NCORES)), trace=trace)
    global LAST_RESULT
    LAST_RESULT = res
    outs = [np.asarray(r["out"], np.float32) for r in res.results]

    colsum = (xf @ consts["wc"]).astype(np.float32)
    bcross = (xf @ consts["bc"]).astype(np.float32)
    return _epilogue(outs, consts, colsum, bcross)


def _epilogue(outs, consts, colsum, bcross):
    # outs: per-core [33, 2, 512]; [0:32,jt,:] = Y^T, [32,jt,:] = SQ
    Y = np.concatenate(
        [o[0:32].reshape(32, JPC).T for o in outs], 0)[:B * N]
    SQ = np.concatenate([o[32].reshape(JPC) for o in outs], 0)[:B * N]

    mu = colsum / np.float32(D) + consts["bmean"]
    E2 = (SQ + 2.0 * bcross + consts["bsq"]) / np.float32(D)
    var = E2 - mu ** 2
    rstd = (1.0 / np.sqrt(var + EPS)).astype(np.float32)
    G = Y + consts["bVg"][None]
    sc = (rstd[:, None] * G
          - (rstd * mu)[:, None] * consts["pg"][None]
          + consts["qb"][None] + consts["c0"][None])
    sc = sc.reshape(B, N, H, 2).transpose(0, 2, 3, 1)       # [B,H,2,N]

    scores = np.empty((B, H, 2, S), np.float32)
    scores[:, :, :, 2:] = sc
    scores[:, :, :, 0:2] = consts["s_spec"][None]

    m = scores - scores.max(-1, keepdims=True)
    e = np.exp(m)
    attn = e / e.sum(-1, keepdims=True)
    mm = attn.mean(1)                                       # [B,2,S]
    return (np.ascontiguousarray(mm[:, 0, :]),
            np.ascontiguousarray(mm[:, 1, :]))


# revision 42
# speedup vs baseline: 1.1250x; 1.1250x over previous
"""Trainium2 Bass kernel for nn_AttentionSiphon.

Reference computes: tokens = x @ W_map + b_map; concat [time, cluster, tokens];
LayerNorm; per-head q/k projections; softmax(q k^T / sqrt(dh)); mean over heads;
returns rows 0 and 1 of the [B,S,S] head-mean attention.

Only attention rows 0/1 are returned, and their queries come from the
(batch-independent) time/cluster tokens, so per-head attention collapses to

  score[j, c=2h+r] = LN(token_j) . (Wk[h] @ q_r[h])   (+ constants)

The 34 score/stat columns are LINEAR in x:  Y = Vaug^T (W^T x^T) = A^T x^T
with A = W @ Vaug [512, 34] precomputed on host.  Only the LayerNorm
sum-of-squares is quadratic:  SQ_j = ||W^T x_j||^2 = x_j^T (W W^T) x_j
= ||L^T x_j||^2 with L = cholesky(W W^T) [512, 512].  So the device work per
core (1024 token columns) is U = L^T x (512-contraction, half the FLOPs of the
naive 1024-wide token projection), squares+reduce for SQ, and the tiny A^T x.
L is lower-triangular, so of the 4x4 grid of [128,128] contraction blocks only
the kc >= dc ones are nonzero: 10 matmuls per 512-column tile instead of 16.

Device output per core: [34, 2, 1024] f32 — [:,0,:] = Y^T, [0,1,:] = SQ.
The tiny softmax epilogue runs on host (identical to the previous scheme).
"""

import os
import sys

sys.path.insert(0, "/opt/trn_rl_repo")

import numpy as np
import ml_dtypes

B, N, IN_D = 4, 2046, 512
D, H, DH = 1024, 16, 64
S = N + 2
EPS = 1e-5
NCORES = 8
JPC = 1024            # padded rows per core
JTOT = NCORES * JPC   # 8192 (8184 real rows + 8 pad)
NAUG = 34             # 32 score cols + colsum + b_map cross
NC_OUT = NAUG + 1     # + sumsq row

# Precision scheme: "bf16" (fastest, ~1.6e-3 rel err),
# "f32r" (fp32-storage reduced-precision matmuls at bf16 PE speed, ~2e-4)
PRECISION = os.environ.get("AS_PRECISION", "bf16")
WARMUP_MMS = int(os.environ.get("AS_WARM", "23"))

_PROG_CACHE = {}
LAST_RESULT = None  # BassKernelResults of the most recent run (for test harness)


def _bf16(a):
    return np.asarray(a, np.float32).astype(ml_dtypes.bfloat16)


def _build_program(precision, warmup=None):
    if warmup is None:
        warmup = WARMUP_MMS
    import concourse.bacc as bacc
    import concourse.mybir as mybir
    from concourse import tile
    from concourse.tile import ScopedClock

    class LeanTailTileContext(tile.TileContext):
        """Skip the exit-path double all-engine barrier + per-sem clears.

        The kernel preamble (Bass.__init__, target_bir_lowering) already
        dma_reset+sem_clears the kernel sem range at the start of every
        execution, and this program has a single TileContext, so nothing
        downstream consumes the freed sems. The final Sync drain still
        waits on every proc (incl. DMA lanes), so outputs are complete
        before the instruction streams end.
        """

        def _drain_and_barrier(self, tick_clock, wait_clock):
            drain_inst = self.nc.sync.drain()
            wait_clock.add_sem_waits(
                drain_inst.ins, ScopedClock({None: tick_clock.global_clock})
            )
            popped = self.nc._tile_sem_poison_stack.pop()
            assert popped is self._sem_poison

    f32 = mybir.dt.float32
    bf16 = mybir.dt.bfloat16
    AF = mybir.ActivationFunctionType

    nc = bacc.Bacc("TRN2")

    bf = mybir.dt.float32r if precision == "f32r" else bf16

    # L-blocks (kc>=dc, per dc in emission order dc=3,2,1,0) + A chunks,
    # all fused into one per-partition-contiguous tensor for a single
    # fat-packet DMA.  Column offsets precomputed here.
    DCS = [3, 2, 1, 0]
    lblk = {}
    col = 0
    for dc in DCS:
        for kc in range(dc, 4):
            lblk[(dc, kc)] = col
            col += 128
    acol = {}
    for kc in range(4):
        acol[kc] = col
        col += 32
    LWA_W = col  # 10*128 + 4*32 = 1408

    # Inputs fused into fat tensors (whole rows DMA'd at once) so each
    # partition row is one long contiguous DRAM region — short rows starve
    # the DMA engines on descriptor fetches (measured 58% vs 100% engine
    # busy).  Split by first-need: weights + x chunks kc3/kc2 unblock the
    # first U groups, kc1/kc0 the rest of jt0, then jt1.
    inA = nc.dram_tensor("inA", [128, LWA_W + 512], bf, kind="ExternalInput")
    inB = nc.dram_tensor("inB", [128, 1536], bf, kind="ExternalInput")
    inC = nc.dram_tensor("inC", [128, 2048], bf, kind="ExternalInput")
    # out[0:32, jt, :] = Y^T (32 scores); out[32, jt, :] = sumsq.  Y and SQ
    # share one [33, 512] PSUM tile per jt — the sumsq ones-matmuls target
    # partition 32 via tile_position=(0, 32) — so each jt needs a single
    # PSUM->SBUF copy, and one fat final DMA ships both jt halves.
    out_h = nc.dram_tensor("out", [33, 2, 512], f32, kind="ExternalOutput")

    ones_bf = nc.const_aps.tensor(1.0, [128, 1], bf16)

    with LeanTailTileContext(nc) as tc:
        with (
            tc.tile_pool(name="cst", bufs=1) as cst,
            tc.tile_pool(name="scr", bufs=2) as scr,
            tc.tile_pool(name="ps_u", bufs=5, space="PSUM") as ps_u,
            tc.tile_pool(name="ps_y", bufs=2, space="PSUM") as ps_y,
            tc.tile_pool(name="ps_w", bufs=1, space="PSUM") as ps_w,
        ):
            inA_sb = cst.tile([128, LWA_W + 512], bf, name="inA_sb",
                              tag="inA")
            inB_sb = cst.tile([128, 1536], bf, name="inB_sb", tag="inB")
            inC_sb = cst.tile([128, 2048], bf, name="inC_sb", tag="inC")
            out_sb = cst.tile([33, 2, 512], f32, name="out_sb")

            def lwa_sl(c, w):
                return inA_sb[:, c:c + w]

            def xt_sl(jt, kc):
                # jt0: kc3 rides with the weights in inA; kc2/kc1/kc0
                # follow in inB in first-need order; jt1 all in inC
                if jt == 1:
                    return inC_sb[:, kc * 512:(kc + 1) * 512]
                if kc == 3:
                    return inA_sb[:, LWA_W:LWA_W + 512]
                return inB_sb[:, (2 - kc) * 512:(3 - kc) * 512]

            # All input DMA on the Sync HWDGE ring: one ring at full rate
            # beats two shared ones, and the Scalar ring stalls ~1.5us
            # behind its activation-table load.
            nc.sync.dma_start(inA_sb[:], inA[:])
            nc.sync.dma_start(inB_sb[:], inB[:])
            nc.sync.dma_start(inC_sb[:], inC[:])

            # PE warm-up during the DMA fill: the HAM activity monitor only
            # un-throttles (1.2 -> 2.4 GHz) after ~3.4us of genuinely busy
            # PE; N=1 matmuls don't register, so stream N=256 ones off a
            # memset tile (baseline-style).
            if warmup:
                warm_sb = cst.tile([128, 256], bf16, name="warm_sb")
                nc.gpsimd.memset(warm_sb[:], 0.25)
                psw = ps_w.tile([128, 256], f32, name="psw", tag="psw")
                for _ in range(warmup):
                    nc.tensor.matmul(psw[:], warm_sb[:, 0:128], warm_sb[:],
                                     start=True, stop=True)

            for jt in range(2):
                # ---- U = L^T x (triangular: block dc needs kc>=dc) ----
                # dc=3 first (1 matmul) so its square lands early; the
                # sumsq ones-matmuls accumulate as squares become ready,
                # with Y before the last one so the PE never stalls.
                sq = {}
                for dc in DCS:
                    psu = ps_u.tile([128, 512], f32, name="psu", tag="psu")
                    kcs = list(range(dc, 4))
                    for ki, kc in enumerate(kcs):
                        nc.tensor.matmul(
                            psu[:],
                            lwa_sl(lblk[(dc, kc)], 128),
                            xt_sl(jt, kc),
                            start=(ki == 0),
                            stop=(ki == len(kcs) - 1),
                        )
                    # squared chunk (bf16; LN variance is error-tolerant)
                    sq_t = scr.tile([128, 512], bf16, name=f"sq{dc}",
                                    tag=f"sq{dc}")
                    nc.scalar.activation(sq_t[:], psu[:], AF.Square)
                    sq[dc] = sq_t

                py = ps_y.tile([33, 512], f32, name="py", tag="py")
                # sumsq partial sums into partition 32 as squares arrive
                for dc in [3, 2, 1]:
                    nc.tensor.matmul(py[32:33, :], ones_bf, sq[dc][:],
                                     start=(dc == 3), stop=False,
                                     tile_position=(0, 32))
                # ---- scores Y^T = A^T x into partitions 0..31 ----
                for kc in range(4):
                    nc.tensor.matmul(
                        py[0:32, :],
                        lwa_sl(acol[kc], 32),
                        xt_sl(jt, kc),
                        start=(kc == 0),
                        stop=(kc == 3),
                    )
                # last sumsq chunk lands while Y streams
                nc.tensor.matmul(py[32:33, :], ones_bf, sq[0][:],
                                 start=False, stop=True,
                                 tile_position=(0, 32))

                nc.vector.tensor_copy(out_sb[:, jt, :], py[:])

            # one fat-row DMA for both halves (thin per-jt slices starve
            # the DMA engines; the jt0 half just waits for jt1's copy)
            nc.sync.dma_start(out_h[:], out_sb[:])

    nc.compile()
    return nc


def _host_precompute(inputs):
    x = np.asarray(inputs["x"], np.float32)
    W = np.asarray(inputs["W_map"], np.float32)
    b_map = np.asarray(inputs["b_map"], np.float32)
    g = np.asarray(inputs["ln_g"], np.float32)
    lb = np.asarray(inputs["ln_b"], np.float32)
    Wq = np.asarray(inputs["Wq"], np.float32)
    bq = np.asarray(inputs["bq"], np.float32)
    Wk = np.asarray(inputs["Wk"], np.float32)
    bk = np.asarray(inputs["bk"], np.float32)
    tt = np.asarray(inputs["time_token"], np.float32)
    ct = np.asarray(inputs["cluster_token"], np.float32)

    spec = np.concatenate([tt, ct], 0)                      # [2, D]
    mu = spec.mean(-1, keepdims=True)
    var = ((spec - mu) ** 2).mean(-1, keepdims=True)
    hspec = ((spec - mu) / np.sqrt(var + EPS) * g + lb).reshape(2, H, DH)
    q = np.einsum("rhd,hde->rhe", hspec, Wq) + bq[None]
    qs = (q / np.sqrt(DH)).astype(np.float32)               # [2,H,DH]
    kspec = np.einsum("rhd,hde->rhe", hspec, Wk) + bk[None]
    s_spec = np.einsum("rhe,the->hrt", qs, kspec)           # [H,2,2]

    v = np.einsum("hde,rhe->hdr", Wk, qs)                   # [H,DH,2]
    V = np.zeros((D, 2 * H), np.float32)
    for h in range(H):
        V[64 * h:64 * h + 64, 2 * h] = v[h, :, 0]
        V[64 * h:64 * h + 64, 2 * h + 1] = v[h, :, 1]
    c0 = np.empty(2 * H, np.float32)
    for h in range(H):
        c0[2 * h] = qs[0, h] @ bk[h]
        c0[2 * h + 1] = qs[1, h] @ bk[h]

    Vg = g[:, None] * V
    consts = dict(
        pg=Vg.sum(0),
        qb=(lb[:, None] * V).sum(0),
        bVg=(b_map[:, None] * Vg).sum(0),
        bmean=b_map.mean(),
        bsq=(b_map ** 2).sum(),
        s_spec=s_spec,
        c0=c0,
        # colsum/bcross are linear in x with tiny [512] maps — cheaper and
        # more accurate on host than as extra device score columns
        wc=(W @ np.ones(D, np.float32)).astype(np.float32),
        bc=(W @ b_map).astype(np.float32),
    )

    # collapse the linear part through W; factor the quadratic part
    W64 = W.astype(np.float64)
    A = (W64 @ Vg.astype(np.float64)).astype(np.float32)    # [512, 32]
    L = np.linalg.cholesky(W64 @ W64.T).astype(np.float32)  # [512, 512]
    return x, A, L, consts


def kernel(**inputs):
    from concourse.bass_utils import run_bass_kernel_spmd

    x, A, L, consts = _host_precompute(inputs)

    key = (PRECISION, WARMUP_MMS)
    if key not in _PROG_CACHE:
        _PROG_CACHE[key] = _build_program(PRECISION, WARMUP_MMS)
    nc = _PROG_CACHE[key]

    cast = (lambda a: np.asarray(a, np.float32)) if PRECISION == "f32r" \
        else _bf16

    xf = x.reshape(B * N, IN_D)
    xpad = np.zeros((JTOT, IN_D), np.float32)
    xpad[:B * N] = xf

    # fused L-blocks (kc>=dc, dc order 3,2,1,0) + A chunks: [128, 1408];
    # x^T jt0's kc3 chunk rides along in inA (fat DMA rows)
    LWA_W = 10 * 128 + 4 * 32
    lwa_np = np.empty((128, LWA_W), np.float32)
    col = 0
    for dc in [3, 2, 1, 0]:
        for kc in range(dc, 4):
            lwa_np[:, col:col + 128] = \
                L[kc * 128:(kc + 1) * 128, dc * 128:(dc + 1) * 128]
            col += 128
    for kc in range(4):
        lwa_np[:, col:col + 32] = A[kc * 128:(kc + 1) * 128, :]
        col += 32
    lwa_c = cast(lwa_np)

    in_maps = []
    for c in range(NCORES):
        xT = np.ascontiguousarray(xpad[c * JPC:(c + 1) * JPC].T)  # [512,1024]
        # [512, 1024] -> [128p, 2jt, 4kc, 512]
        xp = cast(xT).reshape(4, 128, 2, 512).transpose(1, 2, 0, 3)
        ia = np.empty((128, LWA_W + 512), lwa_c.dtype)
        ia[:, :LWA_W] = lwa_c
        ia[:, LWA_W:] = xp[:, 0, 3]                       # jt0 kc3
        ib = np.empty((128, 1536), lwa_c.dtype)
        for kc in (2, 1, 0):                              # first-need order
            ib[:, (2 - kc) * 512:(3 - kc) * 512] = xp[:, 0, kc]
        m = {"inA": np.ascontiguousarray(ia),
             "inB": np.ascontiguousarray(ib),
             "inC": np.ascontiguousarray(xp[:, 1].reshape(128, 2048))}
        in_maps.append(m)

    trace = bool(int(os.environ.get("AS_TRACE", "0")))
    res = run_bass_kernel_spmd(nc, in_maps, list(range(NCORES)), trace=trace)
    global LAST_RESULT
    LAST_RESULT = res
    outs = [np.asarray(r["out"], np.float32) for r in res.results]

    colsum = (xf @ consts["wc"]).astype(np.float32)
    bcross = (xf @ consts["bc"]).astype(np.float32)
    return _epilogue(outs, consts, colsum, bcross)


def _epilogue(outs, consts, colsum, bcross):
    # outs: per-core [33, 2, 512]; [0:32,jt,:] = Y^T, [32,jt,:] = SQ
    Y = np.concatenate(
        [o[0:32].reshape(32, JPC).T for o in outs], 0)[:B * N]
    SQ = np.concatenate([o[32].reshape(JPC) for o in outs], 0)[:B * N]

    mu = colsum / np.float32(D) + consts["bmean"]
    E2 = (SQ + 2.0 * bcross + consts["bsq"]) / np.float32(D)
    var = E2 - mu ** 2
    rstd = (1.0 / np.sqrt(var + EPS)).astype(np.float32)
    G = Y + consts["bVg"][None]
    sc = (rstd[:, None] * G
          - (rstd * mu)[:, None] * consts["pg"][None]
          + consts["qb"][None] + consts["c0"][None])
    sc = sc.reshape(B, N, H, 2).transpose(0, 2, 3, 1)       # [B,H,2,N]

    scores = np.empty((B, H, 2, S), np.float32)
    scores[:, :, :, 2:] = sc
    scores[:, :, :, 0:2] = consts["s_spec"][None]

    m = scores - scores.max(-1, keepdims=True)
    e = np.exp(m)
    attn = e / e.sum(-1, keepdims=True)
    mm = attn.mean(1)                                       # [B,2,S]
    return (np.ascontiguousarray(mm[:, 0, :]),
            np.ascontiguousarray(mm[:, 1, :]))
